# revision 7
# baseline (speedup 1.0000x reference)
"""Strided attention (stride-8 key/value gather) on 8 TRN2 NeuronCores.

Sharding: 8 cores = 4 batches x 2 sequence halves. Each core computes
all 16 heads for its 2048 query tokens against its batch's 512 strided
K/V positions, so there is no cross-core communication at all; the
output slices are disjoint.

Device-side math (per core, all matmuls fp32r = full-rate tf32-like):
  QT[e,t]   = WqT.T @ xqT  (+bq)            [1024 x 2048]
  KT[e,s]   = (0.125*Wk)T.T @ xsT           [1024 x 512]  (k-bias dropped:
              constant along softmax axis; 1/sqrt(64) folded into Wk)
  V[s,e]    = xsT.T @ WvT                   [512 x 1024]  (v-bias folded
              into the output-projection bias on host)
  scoresT   = KT_h.T @ QT_h per head        [512 x 2048]
  expT      = exp(scoresT)                  (no max-subtraction; scores
              are O(5), exp is safe in fp32)
  PV        = [V_h|ones].T @ expT  -> rows = attn_h and the softmax
              denominator r_h replicated, in one matmul
  attnT_h   = attn_h * (1/r_h)   (recip on DVE with a cross-partition AP,
              moving the denominator onto the attn rows)
  outT[e,t] = WoT.T @ attnT (+bo')          [1024 x 2048]
Host reassembles out[b, half] = outT.T.
"""

import sys

if "/opt/trn_rl_repo" not in sys.path:
    sys.path.insert(0, "/opt/trn_rl_repo")

import numpy as np
import orjson

import concourse.bass as bass
import concourse.mybir as mybir
from concourse import tile

# ---------------------------------------------------------------------------
# BIR sync legalizer: the walrus build here accepts at most one sem wait and
# one sem update per instruction, but Tile/bass attach several (tail Drain
# aggregates the global clock; matmul LDWEIGHTS picks up multi-waits).
# Hoist extras onto same-engine EventSemaphore instructions at serialization.
# ---------------------------------------------------------------------------

_DMA_OPCODES = {"DMACopy", "DMATranspose", "TriggeredCopy", "CollectiveCompute"}


def _legalize_module(m: dict) -> bool:
    changed = False
    uid = 0
    for f in m.get("functions", []):
        for b in f.get("blocks", []):
            out = []
            for i in b.get("instructions", []):
                si = i.get("sync_info")
                if not si:
                    out.append(i)
                    continue
                ow = si.get("on_wait") or []
                ou = si.get("on_update") or []
                if len(ow) > 1:
                    for w in ow[:-1]:
                        out.append(
                            {
                                "debug": i.get("debug", 0),
                                "engine": i["engine"],
                                "name": f"{i['name']}-xw{uid}",
                                "opcode": "EventSemaphore",
                                "sync_info": {"on_update": [], "on_wait": [w]},
                            }
                        )
                        uid += 1
                    si["on_wait"] = [ow[-1]]
                    changed = True
                out.append(i)
                if len(ou) > 1:
                    if i.get("opcode") in _DMA_OPCODES:
                        raise RuntimeError(
                            f"multi-update DMA {i['name']} cannot be legalized"
                        )
                    for u in ou[1:]:
                        out.append(
                            {
                                "debug": i.get("debug", 0),
                                "engine": i["engine"],
                                "name": f"{i['name']}-xu{uid}",
                                "opcode": "EventSemaphore",
                                "sync_info": {"on_update": [u], "on_wait": []},
                            }
                        )
                        uid += 1
                    si["on_update"] = [ou[0]]
                    changed = True
            b["instructions"] = out
    return changed


_orig_to_json_bytes = bass.Bass.to_json_bytes


def _patched_to_json_bytes(self) -> bytes:
    raw = _orig_to_json_bytes(self)
    m = orjson.loads(raw)
    if _legalize_module(m):
        raw = orjson.dumps(m)
    return raw


bass.Bass.to_json_bytes = _patched_to_json_bytes

# ---------------------------------------------------------------------------

B, S, E = 4, 4096, 1024
H, D, STRIDE = 16, 64, 8
N_CORES = 8
T = (B * S) // N_CORES  # 2048 query tokens per core
SK = S // STRIDE  # 512 strided keys per batch
KC = E // 128  # 8 embedding chunks
TN = T // 512  # 4 tiles of 512 tokens
F32 = mybir.dt.float32
F32R = mybir.dt.float32r
AF = mybir.ActivationFunctionType


def _r(ap):
    return ap.bitcast(F32R)


def build_nc(reps: int = 1) -> bass.Bass:
    nc = bass.Bass()
    xqT = nc.declare_dram_parameter("xqT", [E, T], F32R, isOutput=False)
    xsT = nc.declare_dram_parameter("xsT", [E, SK], F32R, isOutput=False)
    wqT = nc.declare_dram_parameter("wqT", [E, E], F32R, isOutput=False)
    wkT = nc.declare_dram_parameter("wkT", [E, E], F32R, isOutput=False)
    wvT = nc.declare_dram_parameter("wvT", [E, E], F32R, isOutput=False)
    woT = nc.declare_dram_parameter("woT", [E, E], F32R, isOutput=False)
    bq = nc.declare_dram_parameter("bq", [E, 1], F32, isOutput=False)
    bo2 = nc.declare_dram_parameter("bo2", [E, 1], F32, isOutput=False)
    outT = nc.declare_dram_parameter("outT", [E, T], F32, isOutput=True)

    with tile.TileContext(nc) as tc:
        for _rep in range(reps):
            _emit_body(nc, tc, xqT, xsT, wqT, wkT, wvT, woT, bq, bo2, outT)
    return nc


def _emit_body(nc, tc, xqT, xsT, wqT, wkT, wvT, woT, bq, bo2, outT):
    with tc.tile_pool(name="pers", bufs=1) as pers:
        # ---- persistent tiles: weights, biases, KT, V ----
        wq_sb = [pers.tile([128, E], F32R, name=f"wq{k}") for k in range(KC)]
        wo_sb = [pers.tile([128, E], F32R, name=f"wo{k}") for k in range(KC)]
        kt_sb = [pers.tile([128, SK], F32R, name=f"kt{m}") for m in range(KC)]
        v_sb = [pers.tile([128, H * 128], F32R, name=f"v{j}") for j in range(4)]
        bq_sb = pers.tile([128, KC], F32, name="bq_sb")
        bo_sb = pers.tile([128, KC], F32, name="bo_sb")

        bq_d = bq.rearrange("(c p) one -> c p one", p=128)
        bo_d = bo2.rearrange("(c p) one -> c p one", p=128)
        for m in range(KC):
            nc.sync.dma_start(out=bq_sb[:, m : m + 1], in_=bq_d[m])
            nc.sync.dma_start(out=bo_sb[:, m : m + 1], in_=bo_d[m])
            nc.sync.dma_start(out=wq_sb[m][:], in_=wqT[m * 128 : (m + 1) * 128, :])
            nc.sync.dma_start(out=wo_sb[m][:], in_=woT[m * 128 : (m + 1) * 128, :])

        # ---- stage A: K/V projections for the 512 strided positions ----
        with (
            tc.tile_pool(name="stgA", bufs=1) as stA,
            tc.tile_pool(name="psA", bufs=2, space="PSUM") as psA,
        ):
            xs_sb = [stA.tile([128, SK], F32R, name=f"xs{k}") for k in range(KC)]
            wk_sb = [stA.tile([128, E], F32R, name=f"wk{k}") for k in range(KC)]
            wv_sb = [stA.tile([128, E], F32R, name=f"wv{k}") for k in range(KC)]
            for k in range(KC):
                nc.sync.dma_start(out=xs_sb[k][:], in_=xsT[k * 128 : (k + 1) * 128, :])
                nc.sync.dma_start(out=wk_sb[k][:], in_=wkT[k * 128 : (k + 1) * 128, :])
                nc.sync.dma_start(out=wv_sb[k][:], in_=wvT[k * 128 : (k + 1) * 128, :])

            # KT[e_out, s]: lhsT = wkT chunk, rhs = xsT chunk
            for m in range(KC):
                ps_k = psA.tile([128, SK], F32, name="ps_k", tag="psA")
                for k in range(KC):
                    nc.tensor.matmul(
                        ps_k[:],
                        _r(wk_sb[k][:, m * 128 : (m + 1) * 128]),
                        _r(xs_sb[k][:]),
                        start=(k == 0),
                        stop=(k == KC - 1),
                    )
                nc.vector.tensor_copy(kt_sb[m][:], ps_k[:])

            # V[s, e_out] interleaved per head as [V_h | ones] (even h) and
            # [ones | V_h] (odd h) in v_sb[j][:, h*128:(h+1)*128].
            ones_sc = stA.tile([128, 1024], F32, name="ones_sc")
            nc.gpsimd.memset(ones_sc[:], 1.0)
            ones_v = ones_sc.rearrange("p (q c) -> p q c", c=128)
            for j in range(4):
                # per head pair q: [V_even | ones | ones | V_odd] -> the two
                # ones blocks are contiguous at cols q*256+64..q*256+191
                vE = v_sb[j].rearrange("p (q b) -> p q b", b=256)
                nc.vector.tensor_copy(vE[:, :, 64:192], ones_v)
            for j in range(4):
                v3 = v_sb[j].rearrange("p (h c) -> p h c", c=128)
                for n in range(2):  # e_out halves: heads n*8..n*8+7
                    ps_v = psA.tile([128, 512], F32, name="ps_v", tag="psA")
                    for k in range(KC):
                        nc.tensor.matmul(
                            ps_v[:],
                            _r(xs_sb[k][:, j * 128 : (j + 1) * 128]),
                            _r(wv_sb[k][:, n * 512 : (n + 1) * 512]),
                            start=(k == 0),
                            stop=(k == KC - 1),
                        )
                    pv3 = ps_v.rearrange("p (h d) -> p h d", d=64)
                    for hh in range(8):
                        h = n * 8 + hh
                        col0 = 0 if h % 2 == 0 else 64
                        nc.vector.tensor_copy(
                            v3[:, h : h + 1, col0 : col0 + 64],
                            pv3[:, hh : hh + 1, :],
                        )

        # ---- stage B: per 512-token tile ----
        with (
            tc.tile_pool(name="stgB", bufs=1) as stB,
            tc.tile_pool(name="psQ", bufs=2, space="PSUM") as psQ,
            tc.tile_pool(name="psS", bufs=2, space="PSUM") as psS,
            tc.tile_pool(name="psP", bufs=2, space="PSUM") as psP,
            tc.tile_pool(name="psO", bufs=2, space="PSUM") as psO,
        ):
            for ti in range(TN):
                tsl = slice(ti * 512, (ti + 1) * 512)
                xq_sb = []
                for k in range(KC):
                    xq = stB.tile([128, 512], F32R, name=f"xq{k}", tag=f"xq{k}", bufs=1)
                    nc.sync.dma_start(out=xq[:], in_=xqT[k * 128 : (k + 1) * 128, tsl])
                    xq_sb.append(xq)

                qt_sb = []
                for m in range(KC):
                    ps_q = psQ.tile([128, 512], F32, name="ps_q", tag="psq")
                    for k in range(KC):
                        nc.tensor.matmul(
                            ps_q[:],
                            _r(wq_sb[k][:, m * 128 : (m + 1) * 128]),
                            _r(xq_sb[k][:]),
                            start=(k == 0),
                            stop=(k == KC - 1),
                        )
                    qt = stB.tile([128, 512], F32R, name=f"qt{m}", tag=f"qt{m}", bufs=2)
                    nc.scalar.activation(
                        qt[:], ps_q[:], AF.Identity, bias=bq_sb[:, m : m + 1]
                    )
                    qt_sb.append(qt)

                at_sb = []
                for m in range(KC):
                    at = stB.tile([128, 512], F32R, name=f"at{m}", tag=f"at{m}", bufs=1)
                    at_sb.append(at)

                for hp in range(KC):  # head pair (2*hp, 2*hp+1)
                    pv_ps = []
                    for sub in range(2):
                        h = 2 * hp + sub
                        off = 64 * sub
                        qh = qt_sb[hp][off : off + 64, :]
                        ex_sb = []
                        for j in range(4):
                            ps_s = psS.tile([128, 512], F32, name="ps_s", tag="pss")
                            nc.tensor.matmul(
                                ps_s[:],
                                _r(kt_sb[hp][off : off + 64, j * 128 : (j + 1) * 128]),
                                _r(qh),
                                start=True,
                                stop=True,
                            )
                            ex = stB.tile(
                                [128, 512], F32R, name="ex", tag=f"ex{j}", bufs=2
                            )
                            nc.scalar.activation(ex[:], ps_s[:], AF.Exp)
                            ex_sb.append(ex)
                        ps_pv = psP.tile([128, 512], F32, name="ps_pv", tag="pspv")
                        for j in range(4):
                            nc.tensor.matmul(
                                ps_pv[:],
                                _r(v_sb[j][:, h * 128 : (h + 1) * 128]),
                                _r(ex_sb[j][:]),
                                start=(j == 0),
                                stop=(j == 3),
                            )
                        pv_ps.append(ps_pv)

                    # even head: attn rows 0-63, r rows 64-127
                    # odd head:  r rows 0-63, attn rows 64-127
                    rs = stB.tile([128, 512], F32, name="rs", tag="rs", bufs=2)
                    nc.vector.reciprocal(rs[0:64, :], pv_ps[0][64:128, :])
                    nc.vector.reciprocal(rs[64:128, :], pv_ps[1][0:64, :])
                    at = at_sb[hp]
                    nc.vector.tensor_mul(at[0:64, :], pv_ps[0][0:64, :], rs[0:64, :])
                    nc.vector.tensor_mul(
                        at[64:128, :], pv_ps[1][64:128, :], rs[64:128, :]
                    )

                for me in range(KC):
                    ps_o = psO.tile([128, 512], F32, name="ps_o", tag="pso")
                    for f in range(KC):
                        nc.tensor.matmul(
                            ps_o[:],
                            _r(wo_sb[f][:, me * 128 : (me + 1) * 128]),
                            _r(at_sb[f][:]),
                            start=(f == 0),
                            stop=(f == KC - 1),
                        )
                    ot = stB.tile([128, 512], F32, name="ot", tag="ot", bufs=2)
                    nc.scalar.activation(
                        ot[:], ps_o[:], AF.Identity, bias=bo_sb[:, me : me + 1]
                    )
                    nc.sync.dma_start(
                        out=outT[me * 128 : (me + 1) * 128, tsl], in_=ot[:]
                    )


_NC_CACHE: dict[int, bass.Bass] = {}


def _get_nc(reps: int = 1) -> bass.Bass:
    if reps not in _NC_CACHE:
        _NC_CACHE[reps] = build_nc(reps)
    return _NC_CACHE[reps]


def _host_prep(x, qkv_w, qkv_b, out_w, out_b):
    x = np.asarray(x, dtype=np.float32)
    qkv_w = np.asarray(qkv_w, dtype=np.float32)
    qkv_b = np.asarray(qkv_b, dtype=np.float32)
    out_w = np.asarray(out_w, dtype=np.float32)
    out_b = np.asarray(out_b, dtype=np.float32)

    scale = 1.0 / np.sqrt(D)
    wqT = np.ascontiguousarray(qkv_w[0:E].T)
    wkT = np.ascontiguousarray(qkv_w[E : 2 * E].T * scale)
    wvT = np.ascontiguousarray(qkv_w[2 * E : 3 * E].T)
    woT = np.ascontiguousarray(out_w.T)
    bq = np.ascontiguousarray(qkv_b[0:E].reshape(E, 1))
    bo2 = np.ascontiguousarray(
        (out_b + out_w @ qkv_b[2 * E : 3 * E]).reshape(E, 1)
    )

    in_maps = []
    for c in range(N_CORES):
        b, half = divmod(c, 2)
        xq = x[b, half * T : (half + 1) * T, :]
        xs = x[b, ::STRIDE, :]
        in_maps.append(
            {
                "xqT": np.ascontiguousarray(xq.T),
                "xsT": np.ascontiguousarray(xs.T),
                "wqT": wqT,
                "wkT": wkT,
                "wvT": wvT,
                "woT": woT,
                "bq": bq,
                "bo2": bo2,
            }
        )
    return in_maps


def run(inputs: dict, reps: int = 1):
    from concourse.bass_utils import run_bass_kernel_spmd

    nc = _get_nc(reps)
    in_maps = _host_prep(**inputs)
    res = run_bass_kernel_spmd(nc, in_maps, list(range(N_CORES)))
    out = np.empty((B, S, E), dtype=np.float32)
    for c in range(N_CORES):
        b, half = divmod(c, 2)
        out[b, half * T : (half + 1) * T, :] = res.results[c]["outT"].T
    return out


def kernel(x, qkv_w, qkv_b, out_w, out_b):
    return run(
        {
            "x": x,
            "qkv_w": qkv_w,
            "qkv_b": qkv_b,
            "out_w": out_w,
            "out_b": out_b,
        }
    )
